# revision 42
# baseline (speedup 1.0000x reference)
"""Multi-head attention (B=2, L=2048, H=1024, NH=16) on 8 TRN2 NeuronCores.

Sharding: data-parallel over batch (2) x tensor-parallel over heads (4 groups
of 4 heads).  core = b*4 + g handles batch b, heads [4g, 4g+4).  Wq/Wk/Wv are
split column-wise, Wo row-wise; each core produces a partial [L, H] output
(fp16) that the host sums per batch.

Device math (per core), fp16 matmul inputs / fp32 PSUM accumulation:
  QT = (Wq*0.125)^T x^T          [256, 2048]  (softmax scale folded into Wq)
  KT = Wk^T y^T                  [256, 2048]
  V  = y Wv                      [2048, 256]  lk-partition layout
  heads processed in PAIRS (2p, 2p+1) so the PE's tile_position concurrency
  is exploited twice:
    S^T[lk, lq]: two K=64 matmuls row-tiled at partitions 0-63 / 64-127 run
      concurrently (the qT/kT packing already places head pairs there).
    P^T = exp(S^T) on ScalarE (the pacing engine: 128 x N=1024 exps).
    O^T: two M=64 matmuls col-tiled into one PSUM bank pair (h0 rows 0-63,
      h1 rows 64-127) run concurrently; no ones-row - the softmax
      denominators come from fp16 P-tile accumulation on the DVE plus a
      64-wide ones matmul (also col-tiled) that partition-reduces the
      accumulator, replicated across 64 partitions for the normalize.
  out[lq, 1024] = O'^T_cat^T Wo  partial, stored fp16 (host sums pairs).

PSUM: 3x[128,1024]f32 ring (S pairs, denominator reduces, out-proj pieces)
+ 1x[128,1024]f32 (O-pair accumulator) = 8 banks exactly.  Projections, V,
and the output projection ride inside the exp stream as hook work.
"""

import numpy as np

B, L, H, NH, D = 2, 2048, 1024, 16, 64
GP = 4            # head-groups (tensor-parallel factor)
CH = H // GP      # 256 local projection cols per core
NP = 2            # head pairs per core
LQ = 1024         # lq chunk size
NLQ = L // LQ
NKT = L // 128    # 16 lk tiles
F16 = np.float16

_CACHE = {}


def _build():
    import concourse.mybir as mybir
    import concourse.tile as tile
    from concourse import bacc

    dt = mybir.dt
    f32, fp16 = dt.float32, dt.float16
    Exp = mybir.ActivationFunctionType.Exp

    nc = bacc.Bacc("TRN2", target_bir_lowering=False, debug=False)
    # inputs host-packed partition-major so each DMA is 128 contiguous runs
    xT = nc.declare_dram_parameter("xT", [128, NLQ, 2, 8, 512], fp16,
                                   isOutput=False)
    yT = nc.declare_dram_parameter("yT", [128, NLQ, 2, 8, 512], fp16,
                                   isOutput=False)
    wq = nc.declare_dram_parameter("wq", [128, 2, 8, 128], fp16,
                                   isOutput=False)
    wk = nc.declare_dram_parameter("wk", [128, 2, 8, 128], fp16,
                                   isOutput=False)
    wv = nc.declare_dram_parameter("wv", [128, 8, CH], fp16, isOutput=False)
    wo = nc.declare_dram_parameter("wo", [128, 2, H], fp16, isOutput=False)
    out = nc.declare_dram_parameter("out", [L, H], fp16, isOutput=True)

    with tile.TileContext(nc) as tc:
        with (
            tc.tile_pool(name="w", bufs=1) as wpool,
            tc.tile_pool(name="acts", bufs=1) as apool,
            tc.tile_pool(name="psA", bufs=3, space="PSUM") as psA,
            tc.tile_pool(name="psO", bufs=1, space="PSUM") as psO,
            tc.tile_pool(name="pt", bufs=12) as ptpool,
            tc.tile_pool(name="accp", bufs=4) as accpool,
            tc.tile_pool(name="oT", bufs=1) as otpool,
            tc.tile_pool(name="sm", bufs=2) as smpool,
            tc.tile_pool(name="osb", bufs=4) as opool,
        ):
            ones64 = wpool.tile([128, 64], fp16, tag="ones64")
            nc.vector.memset(ones64, 1.0)
            warm = wpool.tile([128, 512], fp16, tag="warm")
            nc.vector.memset(warm, 0.0)

            # ---- input DMAs, priority order: wv first (V groups pre-run
            # in the idle PE window), then the first-exp gate (wk/wq ct0,
            # y rows 0:512, x rows 0:2048), then by ride deadline ----------
            wk_sb = wpool.tile([128, 2, 8, 128], fp16, tag="wk")
            nc.sync.dma_start(wk_sb[:, 0], wk[:, 0])
            wq_sb = wpool.tile([128, 2, 8, 128], fp16, tag="wq")
            nc.sync.dma_start(wq_sb[:, 0], wq[:, 0])
            yT_sb = apool.tile([128, NLQ, 2, 8, 512], fp16, tag="yT")
            xT_sb = apool.tile([128, NLQ, 2, 8, 512], fp16, tag="xT")
            nc.sync.dma_start(yT_sb[:, 0, 0], yT[:, 0, 0])
            wv_sb = wpool.tile([128, 8, CH], fp16, tag="wv")
            nc.sync.dma_start(wv_sb, wv[:, :, :])
            nc.sync.dma_start(xT_sb[:, 0, 0], xT[:, 0, 0])
            nc.sync.dma_start(xT_sb[:, 0, 1], xT[:, 0, 1])
            nc.sync.dma_start(yT_sb[:, 0, 1], yT[:, 0, 1])
            nc.sync.dma_start(wk_sb[:, 1], wk[:, 1])
            nc.sync.dma_start(wq_sb[:, 1], wq[:, 1])
            for sl in range(2):
                nc.sync.dma_start(yT_sb[:, 1, sl], yT[:, 1, sl])
            for sl in range(2):
                nc.sync.dma_start(xT_sb[:, 1, sl], xT[:, 1, sl])
            wo_sb = wpool.tile([128, 2, H], fp16, tag="wo")
            nc.sync.dma_start(wo_sb, wo[:, :, :])

            # prefetch the exp activation table while input DMAs run (after
            # the dma_starts so the table DMA doesn't delay the DIRECT2Ds)
            dummy = smpool.tile([1, 8], f32, tag="dummy")
            nc.vector.memset(dummy, 0.0)
            nc.scalar.activation(dummy, dummy, Exp)

            # HAM warm-up: keep the PE busy through the DMA window so the
            # first projections run at 2.4 GHz, not the cold 1.2
            wps = psA.tile([128, LQ], f32, tag="psA", name="warmps")
            for _ in range(22):
                nc.tensor.matmul(wps[0:64, 0:512], lhsT=ones64, rhs=warm,
                                 start=True, stop=True)

            qT_sb = apool.tile([128, 2, L], fp16, tag="qT")
            kT_sb = apool.tile([128, 2, L], fp16, tag="kT")
            v_sb = apool.tile([128, NKT, CH], fp16, tag="v")

            def proj_group(w_sb, act_sb, dst, ct, lh, sl):
                # dst[:, ct, lh*LQ+sl*512 : +512] via one 8-matmul psum group
                ps = psA.tile([128, LQ], f32, tag="psA", name="projps")
                off = lh * LQ + sl * 512
                for ht in range(8):
                    nc.tensor.matmul(
                        ps[:, 0:512],
                        lhsT=w_sb[:, ct, ht, :],
                        rhs=act_sb[:, lh, sl, ht, :],
                        start=(ht == 0), stop=(ht == 7),
                    )
                nc.vector.tensor_copy(dst[:, ct, off:off + 512], ps[:, 0:512])

            def v_group(lkt):
                # one lk tile of V[lk, 256]
                psv = psA.tile([128, LQ], f32, tag="psA", name="psv")
                for ht in range(8):
                    nc.tensor.matmul(
                        psv[:, :CH],
                        lhsT=yT_sb[:, lkt // 8, (lkt % 8) // 4, ht,
                                   (lkt % 4) * 128:(lkt % 4 + 1) * 128],
                        rhs=wv_sb[:, ht, :],
                        start=(ht == 0), stop=(ht == 7),
                    )
                nc.vector.tensor_copy(v_sb[:, lkt, :], psv[:, :CH])

            def s3_piece(ci, oT_sb, mt, pool=None, act_copy=False,
                         kts=(0, 1), dst=None):
                # dst[mt*128 : +128, :] = oT[:, kts]^T @ Wo[kts], fp16
                pool = pool if pool is not None else psA
                pso = pool.tile([128, LQ], f32,
                                tag="psA" if pool is psA else "psO",
                                name="s3pso")
                for nt in range(2):
                    for i, kt in enumerate(kts):
                        nc.tensor.matmul(
                            pso[:, nt * 512:(nt + 1) * 512],
                            lhsT=oT_sb[:, kt, mt * 128:(mt + 1) * 128],
                            rhs=wo_sb[:, kt, nt * 512:(nt + 1) * 512],
                            start=(i == 0), stop=(i == len(kts) - 1),
                        )
                osb = opool.tile([128, LQ], fp16, tag="osb")
                if act_copy:
                    nc.scalar.copy(osb, pso)
                else:
                    nc.vector.tensor_copy(osb, pso)
                if dst is None:
                    dst = out[ci * LQ + mt * 128:ci * LQ + (mt + 1) * 128, :]
                nc.sync.dma_start(dst, osb)

            def emit_S_pair(p, ci, lkt):
                # two K=64 matmuls per sl, row-tiled (partitions 0-63 vs
                # 64-127) so each adjacent pair runs concurrently on the PE
                psS = [psA.tile([128, LQ], f32, tag="psA", name=f"psS{h}")
                       for h in range(2)]
                for sl in range(2):
                    for h in range(2):
                        po = slice(64 * h, 64 * h + 64)
                        nc.tensor.matmul(
                            psS[h][:, sl * 512:(sl + 1) * 512],
                            lhsT=kT_sb[po, p, lkt * 128:(lkt + 1) * 128],
                            rhs=qT_sb[po, p,
                                      ci * LQ + sl * 512:
                                      ci * LQ + (sl + 1) * 512],
                            start=True, stop=True,
                        )
                return psS

            pipe = {}

            def s2_pair(p, ci, oT_sb, extra=None, nxt=None):
                # one head pair x one lq chunk: 16 lkt periods of
                # exp x2 -> S(k+1) pair -> acc adds -> hook -> O(k) pair
                psO_c = psO.tile([128, LQ], f32, tag="psO", name="psOc")
                # col-tiled pair shares banks: zero data, then accumulate
                # with start=False throughout (a start=True bank-wide bit
                # clear races with the concurrent tile's writes)
                nc.vector.memset(psO_c, 0.0)
                acc = [accpool.tile([128, LQ], fp16, tag="acc",
                                    name=f"acc{h}") for h in range(2)]
                psS = pipe.pop("psS", None)
                if psS is None:
                    psS = emit_S_pair(p, ci, 0)
                sums = None

                def sums_mms(rhs_of, start_grp, stop_grp):
                    # ones64^T @ rhs -> [64, lq] replicated, col-tiled into
                    # one psA tile (h0 rows 0-63, h1 64-127)
                    for hs in range(2):
                        for h in range(2):
                            nc.tensor.matmul(
                                sums[h * 64:(h + 1) * 64,
                                     hs * 512:(hs + 1) * 512],
                                lhsT=ones64,
                                rhs=rhs_of(h)[:, hs * 512:(hs + 1) * 512],
                                start=False,
                                stop=(stop_grp and h == 1),
                                skip_group_check=True,
                            )

                for lkt in range(NKT):
                    if lkt == NKT - 1:
                        # partial denominator reduce of acc(0..14) rides
                        # here so only pt(15)'s reduce trails the last exp
                        sums = psA.tile([128, LQ], f32, tag="psA",
                                        name="sums")
                        nc.vector.memset(sums, 0.0)
                        sums_mms(lambda h: acc[h], True, False)
                    pt = [ptpool.tile([128, LQ], fp16, tag="pt",
                                      name=f"pt{h}") for h in range(2)]
                    for h in range(2):
                        nc.scalar.activation(pt[h], psS[h], Exp)
                    if lkt + 1 < NKT:
                        psS = emit_S_pair(p, ci, lkt + 1)
                    elif nxt is not None:
                        pipe["psS"] = emit_S_pair(nxt[0], nxt[1], 0)
                    if lkt == NKT - 1:
                        sums_mms(lambda h: pt[h], False, True)
                    else:
                        for h in range(2):
                            if lkt == 0:
                                nc.vector.tensor_copy(acc[h], pt[h])
                            else:
                                nc.vector.tensor_add(acc[h], acc[h], pt[h])
                    # O pair: col-tiled, h0 -> rows 0-63, h1 -> rows 64-127
                    for sl in range(2):
                        for h in range(2):
                            nc.tensor.matmul(
                                psO_c[h * 64:(h + 1) * 64,
                                      sl * 512:(sl + 1) * 512],
                                lhsT=v_sb[:, lkt,
                                          p * 128 + h * 64:
                                          p * 128 + (h + 1) * 64],
                                rhs=pt[h][:, sl * 512:(sl + 1) * 512],
                                start=False,
                                stop=(lkt == NKT - 1 and h == 1),
                                skip_group_check=True,
                            )
                    if extra is not None:
                        extra(lkt)
                rcp = smpool.tile([128, LQ], f32, tag="rcp")
                for hs in range(2):
                    c = slice(hs * 512, (hs + 1) * 512)
                    nc.vector.reciprocal_approx_fast(rcp[:, c], sums[:, c])
                    nc.vector.tensor_mul(
                        oT_sb[:, p, c], psO_c[:, c], rcp[:, c])

            # ---- emission order ------------------------------------------
            oT = [otpool.tile([128, 2, LQ], fp16, tag="oT", name=f"oT{i}")
                  for i in range(NLQ)]
            # startup: what the first exps need; V(0..3) depend only on
            # wv + y rows 0:512 so they run before the x-gated Q groups
            proj_group(wk_sb, yT_sb, kT_sb, 0, 0, 0)
            for j in range(4):
                v_group(j)
            proj_group(wq_sb, xT_sb, qT_sb, 0, 0, 0)
            proj_group(wq_sb, xT_sb, qT_sb, 0, 0, 1)

            def make_hook(sched):
                def hook(lkt):
                    for job in sched.get(lkt, ()):
                        job()
                return hook

            vj = [(lambda j=j: v_group(j)) for j in range(NKT)]
            pj = lambda w, a, d, ct, lh, sl: (  # noqa: E731
                lambda: proj_group(w, a, d, ct, lh, sl))

            def pj2(w_sb, act_sb, dst, ct, lh, sl):
                # one projection psum group split into two 4-matmul thunks
                cell = {}

                def half(r):
                    def thunk():
                        if r == 0:
                            cell["ps"] = psA.tile(
                                [128, LQ], f32, tag="psA",
                                name=f"pjps{ct}_{lh}_{sl}_{id(w_sb) % 97}")
                        ps = cell["ps"]
                        for ht in range(4 * r, 4 * r + 4):
                            nc.tensor.matmul(
                                ps[:, 0:512],
                                lhsT=w_sb[:, ct, ht, :],
                                rhs=act_sb[:, lh, sl, ht, :],
                                start=(ht == 0), stop=(ht == 7),
                            )
                        if r == 1:
                            nc.vector.tensor_copy(
                                dst[:, ct, lh * LQ + sl * 512:
                                    lh * LQ + (sl + 1) * 512], ps[:, 0:512])
                    return thunk
                return half(0), half(1)

            def sched_projs(sched, groups, slot_pairs):
                for g, (sa, sb_) in zip(groups, slot_pairs):
                    a, b = pj2(*g)
                    sched.setdefault(sa, []).append(a)
                    sched.setdefault(sb_, []).append(b)

            # P0C0: V(4..15) staggered ahead of the O stream; K ct0
            # remainder by deadline; prep P1C0 (K/Q ct1 first slices)
            h0_sched = {}
            for j in range(4, NKT):
                h0_sched.setdefault(j - 4, []).append(vj[j])
            sched_projs(h0_sched,
                        [(wk_sb, yT_sb, kT_sb, 0, 0, 1),
                         (wk_sb, yT_sb, kT_sb, 0, 1, 0),
                         (wk_sb, yT_sb, kT_sb, 0, 1, 1),
                         (wk_sb, yT_sb, kT_sb, 1, 0, 0),
                         (wq_sb, xT_sb, qT_sb, 1, 0, 0),
                         (wq_sb, xT_sb, qT_sb, 1, 0, 1)],
                        [(1, 2), (3, 4), (5, 6), (7, 8), (9, 10), (11, 12)])
            # P1C0: K ct1 remainder + Q ct0 chunk1 prep
            h1_sched = {}
            sched_projs(h1_sched,
                        [(wk_sb, yT_sb, kT_sb, 1, 0, 1),
                         (wk_sb, yT_sb, kT_sb, 1, 1, 0),
                         (wk_sb, yT_sb, kT_sb, 1, 1, 1),
                         (wq_sb, xT_sb, qT_sb, 0, 1, 0),
                         (wq_sb, xT_sb, qT_sb, 0, 1, 1)],
                        [(0, 1), (2, 3), (4, 5), (7, 8), (10, 11)])
            # P0C1: s3(C0) pieces + Q ct1 chunk1 prep (k13-15 kept clean
            # so the pair-chunk transition is not congested)
            h2_sched = {}
            sched_projs(h2_sched,
                        [(wq_sb, xT_sb, qT_sb, 1, 1, 0),
                         (wq_sb, xT_sb, qT_sb, 1, 1, 1)],
                        [(0, 4), (2, 6)])
            for i, mt in enumerate(range(8)):
                h2_sched.setdefault([1, 3, 5, 7, 9, 10, 11, 12][i],
                                    []).append(
                    lambda mt=mt: s3_piece(0, oT[0], mt))

            s2_pair(0, 0, oT[0], extra=make_hook(h0_sched), nxt=(1, 0))
            s2_pair(1, 0, oT[0], extra=make_hook(h1_sched), nxt=(0, 1))
            s2_pair(0, 1, oT[1], extra=make_hook(h2_sched), nxt=(1, 1))
            s2_pair(1, 1, oT[1])
            # tail: out-projection of chunk 1 (psA ring is free now; psO
            # frees after the last normalize); alternate the psum->sbuf
            # casts between DVE and ScalarE so they pipeline
            for mt in range(8):
                s3_piece(1, oT[1], mt,
                         pool=(psO if mt % 3 == 2 else psA),
                         act_copy=bool(mt % 2))
    nc.compile()
    return nc


def _get_nc():
    if "nc" not in _CACHE:
        _CACHE["nc"] = _build()
    return _CACHE["nc"]


def _pack_pm(a, t):
    # [t*128, N] -> [128, t, N] partition-major
    return a.reshape(t, 128, -1).transpose(1, 0, 2)


def _pack_act(a):
    # x[b] [L, H] -> xT packed [128, NLQ(lh), 2(sl), 8(t), 512] fp16
    v = _pack_pm(np.ascontiguousarray(a.T), 8)          # [128, 8, L]
    v = v.reshape(128, 8, NLQ, 2, 512).transpose(0, 2, 3, 1, 4)
    return np.ascontiguousarray(v).astype(F16)


def _pack_w(a):
    # W-shard [H, CH] -> [128, 2(ct), 8(t), 128] fp16
    v = _pack_pm(a, 8)                                  # [128, 8, CH]
    v = v.reshape(128, 8, 2, 128).transpose(0, 2, 1, 3)
    return np.ascontiguousarray(v).astype(F16)


def _in_maps(x, y, Wq, Wk, Wv, Wo):
    maps = []
    for core in range(8):
        b, g = core // GP, core % GP
        cs = slice(g * CH, (g + 1) * CH)
        maps.append({
            "xT": _pack_act(x[b]),
            "yT": _pack_act(y[b]),
            "wq": _pack_w(Wq[:, cs] * np.float32(0.125)),
            "wk": _pack_w(Wk[:, cs]),
            "wv": np.ascontiguousarray(_pack_pm(Wv[:, cs], 8)).astype(F16),
            "wo": np.ascontiguousarray(_pack_pm(Wo[cs, :], 2)).astype(F16),
        })
    return maps


def _install_ntff_hook():
    """Provide the antenv.axon_hooks shim missing from this container so
    run_bass_kernel_spmd(trace=True) can drive NTFF profiling via ctypes."""
    import sys
    import types
    try:
        from antenv.axon_hooks import get_axon_ntff_profile_hook  # noqa: F401
        return
    except ImportError:
        pass
    from trn_agent_boot.trn_boot import _ntff_profile_via_ctypes
    hook = _ntff_profile_via_ctypes("/opt/axon/libaxon_pjrt.so")
    mod = types.ModuleType("antenv.axon_hooks")
    mod.get_axon_ntff_profile_hook = lambda: hook
    mod.set_axon_ntff_profile_hook = lambda h: None
    sys.modules["antenv.axon_hooks"] = mod


def _run(inputs, trace=False):
    from concourse import bass_utils

    if trace:
        _install_ntff_hook()

    x, y, bias = inputs["x"], inputs["y"], inputs["bias"]
    if np.count_nonzero(np.asarray(bias)):
        raise NotImplementedError("nonzero attention bias not supported")
    nc = _get_nc()
    maps = _in_maps(np.asarray(x, np.float32), np.asarray(y, np.float32),
                    np.asarray(inputs["Wq"], np.float32),
                    np.asarray(inputs["Wk"], np.float32),
                    np.asarray(inputs["Wv"], np.float32),
                    np.asarray(inputs["Wo"], np.float32))
    res = bass_utils.run_bass_kernel_spmd(
        nc, maps, list(range(8)), trace=trace)
    out = np.zeros((B, L, H), np.float32)
    for core in range(8):
        out[core // GP] += res.results[core]["out"].astype(np.float32)
    return out, res


def kernel(**inputs):
    out, _ = _run(inputs, trace=False)
    return out
